# revision 43
# baseline (speedup 1.0000x reference)
"""Trainium2 Bass kernel: causal multi-head attention (B=2, N=2048, C=2048, 16 heads).

Sharding: 16 heads split across 8 cores (2 heads/core, tensor parallel).
Each core computes q/k/v projections for its 2 heads, causal attention,
and its partial out-projection y_c = ctx_c @ wo_c.T. Host sums partials + bo.

Layout/dtype strategy (vs an all-f32r version, ~430us -> ~311us):
  q/k projections run as pure-fp8e4m3 DoubleRow matmuls (2 c-tiles per
  instruction, ~1.87x PE rate on HW). Operands are cast host-side
  (ml_dtypes) so the device error equals the host-simulated error
  (1.43e-2 < 2e-2 gate): weights are pre-scaled by 64 to clear the
  e4m3 subnormal range and descaled in the PSUM->SBUF activation.
  v projection, scores, AV, row sums and out-proj run in bf16 (same PE
  rate as f32r, adds only ~3.5e-3 err). Output is bf16 (halves output
  DMA); host sums partials in f64.

Per-core layout:
  qT/kT: [head_dim(128) partitions, tokens] bf16  (from lhsT=w^T, rhs=x^T)
  vT is PE-transposed (f32) to V natural [tok, d], stored bf16
  S^T[k, q] = K^T.T @ Q^T tiles (contraction over head_dim), bf16 operands
  E^T = exp(scale * S^T) bf16 (no max subtraction -- scores are ~N(0,1/9))
  ctx^T[d, q] = V.T @ E^T (bf16); row sums via all-ones-lhsT matmuls
  (reduce+broadcast in PSUM), normalized on DVE with fast reciprocal
  y[tok, f] = ctx^T.T @ wo^T (bf16 operands, bf16 output)

Schedule notes (what the ~88% PE occupancy comes from):
  - Phase 1 runs the fp8 q/k sweep before the bf16 v sweep each chunk,
    with DMA issue order matched (few, large transfers: the sync engine
    issues DMAs at ~650ns each, pacing the kernel start).
  - Causality at tile granularity (k-tile <= q-tile), and the 4
    diagonal k-tiles are computed at 256-wide granularity: q-half j2
    needs k-tiles d0..d0+2*j2+1 only (-25% diagonal PE work); partial
    tiles are masked by 0/1 mask multiply after exp.
  - The attention inner loop is paced by the scalar-engine exp
    (~686ns/tile vs ~645ns of PE work), so each group's out-projection
    is software-pipelined one group behind and its PE-heavy matmuls
    fill the scalar-paced slack; ascending qc keeps the first
    (unfillable) group small.
  - Output y streams out per-2-fc-block DMAs to avoid a tail burst.
"""

import os
import numpy as np
import ml_dtypes

import concourse.bass as bass
import concourse.tile as tile
from concourse import bacc, mybir
from concourse import bass_utils

F32 = mybir.dt.float32
BF16 = mybir.dt.bfloat16
FP8 = mybir.dt.float8e4
AF = mybir.ActivationFunctionType
DR = mybir.MatmulPerfMode.DoubleRow

NP_BF16 = ml_dtypes.bfloat16
NP_FP8 = ml_dtypes.float8_e4m3

# problem dims (hardcoded per contract)
B = 2
N = 2048
C = 2048
HEADS = 16
HD = 128          # head dim
NCORES = 8
HPC = HEADS // NCORES  # heads per core = 2
E = HPC * HD      # per-core projection width = 256
BN = B * N        # 4096
P = 128
CT = C // P       # 16 contraction tiles
NCH = 512         # n-chunk width for projections
NCHUNKS = BN // NCH   # 8
QCW = 512         # q-chunk width in attention
QCHUNKS = N // QCW    # 4 per batch
KT_PER_B = N // P     # 16 k-tiles per batch
TOK_TILES = BN // P   # 32
SCALE = float(HD) ** -0.5
WSCALE = 64.0     # host pre-scale on wq/wk to clear e4m3 subnormals
XQ = 4            # x streamed in quarters of 4 c-tiles

_CACHE = {}


def _build():
    nc = bacc.Bacc(
        "TRN2",
        target_bir_lowering=False,
        debug=False,
        enable_asserts=False,
        num_devices=NCORES,
    )

    x8T = nc.dram_tensor("x8T", [C, BN], FP8, kind="ExternalInput").ap()
    xbT = nc.dram_tensor("xbT", [C, BN], BF16, kind="ExternalInput").ap()
    wqk8 = nc.dram_tensor("wqk8", [C, 2 * E], FP8, kind="ExternalInput").ap()
    wvT = nc.dram_tensor("wvT", [C, E], BF16, kind="ExternalInput").ap()
    woT = nc.dram_tensor("woT", [E, C], BF16, kind="ExternalInput").ap()
    bqkv = nc.dram_tensor("bqkv", [3 * HPC, P], F32, kind="ExternalInput").ap()
    masks = nc.dram_tensor("masks", [4, P, QCW], BF16, kind="ExternalInput").ap()
    ones_d = nc.dram_tensor("ones_d", [P, P], BF16, kind="ExternalInput").ap()
    ident_d = nc.dram_tensor("ident_d", [P, P], F32, kind="ExternalInput").ap()
    yp = nc.dram_tensor("yp", [BN, C], BF16, kind="ExternalOutput").ap()

    with tile.TileContext(nc) as tc:
        with tc.tile_pool(name="persist", bufs=1) as persist:
            # persistent per-core activations
            qT = persist.tile([P, HPC, B, N], BF16, tag="qT")
            kT = persist.tile([P, HPC, B, N], BF16, tag="kT")
            vN = persist.tile([P, TOK_TILES, E], BF16, tag="vN")
            masks_sb = persist.tile([P, 4, QCW], BF16, tag="masks")
            ones_sb = persist.tile([P, P], BF16, tag="ones")
            ident_sb = persist.tile([P, P], F32, tag="ident")

            # ---------------- Phase 1: projections ----------------
            with tc.tile_pool(name="p1w", bufs=1) as wpool, \
                 tc.tile_pool(name="p1x8", bufs=8) as x8pool, \
                 tc.tile_pool(name="p1xb", bufs=8) as xbpool, \
                 tc.tile_pool(name="p1vt", bufs=2) as vtpool, \
                 tc.tile_pool(name="p1_ps", bufs=6, space="PSUM") as pps, \
                 tc.tile_pool(name="p1t_ps", bufs=2, space="PSUM") as tps:
                wqk_sb = wpool.tile([P, CT, 2 * E], FP8, tag="wqk")
                wv_sb = wpool.tile([P, CT, E], BF16, tag="wv")
                b_sb = wpool.tile([P, 3 * HPC], F32, tag="bqkv")

                # DMA priority, matched to the new consumption order:
                # q/k weights + chunk-0 x8 pieces first (the qk DR sweep
                # starts within ~2us), then wv + xb pieces (v sweep runs
                # while they land), then phase-2 constants. Few, large
                # transfers: the sync engine issues DMAs at ~650ns each,
                # which paces the kernel start.
                nc.sync.dma_start(b_sb[:], bqkv.rearrange("h p -> p h"))
                wqk8r = wqk8.rearrange("(t p) e -> p t e", p=P)
                wvTr = wvT.rearrange("(t p) e -> p t e", p=P)
                x8Tr = x8T.rearrange("(t p) n -> p t n", p=P)
                xbTr = xbT.rearrange("(t p) n -> p t n", p=P)
                x8h0 = []
                xbh0 = []
                for piece in range(XQ):
                    sl = slice(piece * 4, (piece + 1) * 4)
                    nc.sync.dma_start(wqk_sb[:, sl, :], wqk8r[:, sl, :])
                    x8c = x8pool.tile([P, 4, NCH], FP8, tag="x8c")
                    nc.sync.dma_start(x8c[:], x8Tr[:, sl, 0:NCH])
                    x8h0.append(x8c)
                for piece in range(XQ):
                    sl = slice(piece * 4, (piece + 1) * 4)
                    nc.sync.dma_start(wv_sb[:, sl, :], wvTr[:, sl, :])
                    xbc = xbpool.tile([P, 4, NCH], BF16, tag="xbc")
                    nc.sync.dma_start(xbc[:], xbTr[:, sl, 0:NCH])
                    xbh0.append(xbc)
                nc.sync.dma_start(masks_sb[:], masks.rearrange("a p n -> p a n"))
                nc.sync.dma_start(ones_sb[:], ones_d)
                nc.sync.dma_start(ident_sb[:], ident_d)

                for ch in range(NCHUNKS):
                    b = ch // (N // NCH)
                    nn0 = (ch % (N // NCH)) * NCH  # within-batch token offset
                    n0 = ch * NCH                  # global token offset
                    if ch == 0:
                        x8h, xbh = x8h0, xbh0
                    else:
                        x8h, xbh = [], []
                        for piece in range(XQ):
                            x8c = x8pool.tile([P, 4, NCH], FP8, tag="x8c")
                            nc.sync.dma_start(
                                x8c[:], x8Tr[:, piece * 4:(piece + 1) * 4,
                                             n0:n0 + NCH])
                            x8h.append(x8c)
                        for piece in range(XQ):
                            xbc = xbpool.tile([P, 4, NCH], BF16, tag="xbc")
                            nc.sync.dma_start(
                                xbc[:], xbTr[:, piece * 4:(piece + 1) * 4,
                                             n0:n0 + NCH])
                            xbh.append(xbc)

                    # accumulators: q,k (fp8 DoubleRow) + v (bf16) x 2 heads
                    accq = [pps.tile([P, NCH], F32, tag="pacc",
                                     name=f"paccq_{ch}_{h}") for h in range(HPC)]
                    acck = [pps.tile([P, NCH], F32, tag="pacc",
                                     name=f"pacck_{ch}_{h}") for h in range(HPC)]
                    accv = [pps.tile([P, NCH], F32, tag="pacc",
                                     name=f"paccv_{ch}_{h}") for h in range(HPC)]
                    # qk DoubleRow sweep first (x8 pieces arrive first)
                    for cp in range(CT // 2):
                        x8q = x8h[cp // 2][:, (cp % 2) * 2:(cp % 2) * 2 + 2, :]
                        st = (cp == 0)
                        sp = (cp == CT // 2 - 1)
                        for h in range(HPC):
                            nc.tensor.matmul(
                                accq[h][:],
                                wqk_sb[:, 2 * cp:2 * cp + 2,
                                       h * HD:(h + 1) * HD],
                                x8q, start=st, stop=sp, perf_mode=DR)
                            nc.tensor.matmul(
                                acck[h][:],
                                wqk_sb[:, 2 * cp:2 * cp + 2,
                                       E + h * HD:E + (h + 1) * HD],
                                x8q, start=st, stop=sp, perf_mode=DR)
                    for h in range(HPC):
                        nc.scalar.activation(
                            qT[:, h, b, nn0:nn0 + NCH], accq[h][:],
                            AF.Identity, bias=b_sb[:, h:h + 1],
                            scale=1.0 / WSCALE)
                        nc.scalar.activation(
                            kT[:, h, b, nn0:nn0 + NCH], acck[h][:],
                            AF.Identity, bias=b_sb[:, HPC + h:HPC + h + 1],
                            scale=1.0 / WSCALE)
                    # v sweep (bf16 plain matmuls; xb pieces land during qk)
                    for cp in range(CT // 2):
                        for h in range(HPC):
                            for sub in range(2):
                                ct = 2 * cp + sub
                                nc.tensor.matmul(
                                    accv[h][:],
                                    wv_sb[:, ct, h * HD:(h + 1) * HD],
                                    xbh[ct // 4][:, ct % 4, :],
                                    start=(ct == 0), stop=(ct == CT - 1))
                    for h in range(HPC):
                        # v with bias (f32), then PE-transpose to V natural
                        vt = vtpool.tile([P, NCH], F32, tag="vt")
                        nc.scalar.activation(
                            vt[:], accv[h][:],
                            AF.Identity,
                            bias=b_sb[:, 2 * HPC + h:2 * HPC + h + 1],
                            scale=1.0)
                        for ts in range(NCH // P):
                            tp = tps.tile([P, P], F32, tag="tp")
                            nc.tensor.transpose(
                                tp[:], vt[:, ts * P:(ts + 1) * P], ident_sb[:])
                            nc.vector.tensor_copy(
                                vN[:, ch * (NCH // P) + ts,
                                   h * HD:(h + 1) * HD],
                                tp[:])

            # ---------------- Phase 2: attention + out-proj ----------------
            with tc.tile_pool(name="p2const", bufs=1) as cpool, \
                 tc.tile_pool(name="p2e", bufs=8) as epool, \
                 tc.tile_pool(name="p2ctx", bufs=6) as ctxpool, \
                 tc.tile_pool(name="p2sm", bufs=3) as smpool, \
                 tc.tile_pool(name="p2y", bufs=3) as ysbpool, \
                 tc.tile_pool(name="p2s_ps", bufs=2, space="PSUM") as spool, \
                 tc.tile_pool(name="p2c_ps", bufs=2, space="PSUM") as cps, \
                 tc.tile_pool(name="p2sb_ps", bufs=2, space="PSUM") as sbps, \
                 tc.tile_pool(name="p2y_ps", bufs=2, space="PSUM") as yps:
                wo_sb = cpool.tile([P, HPC, C], BF16, tag="wo")
                nc.sync.dma_start(wo_sb[:], woT.rearrange("(h p) f -> p h f", p=P))

                def outproj(ctx_tiles, b, qc, nts):
                    for nt in nts:
                        y_sb = ysbpool.tile([P, C], BF16, tag="ysb")
                        row0 = b * N + qc * QCW + nt * P
                        for fc in range(C // 512):
                            y_ps = yps.tile([P, 512], F32, tag="yps")
                            for h in range(HPC):
                                nc.tensor.matmul(
                                    y_ps[:],
                                    ctx_tiles[h][:, nt * P:(nt + 1) * P],
                                    wo_sb[:, h, fc * 512:(fc + 1) * 512],
                                    start=(h == 0), stop=(h == HPC - 1),
                                )
                            if fc % 2 == 0:
                                nc.vector.tensor_copy(
                                    y_sb[:, fc * 512:(fc + 1) * 512],
                                    y_ps[:])
                            else:
                                nc.scalar.copy(
                                    y_sb[:, fc * 512:(fc + 1) * 512],
                                    y_ps[:])
                                # drain output as it is produced: halves
                                # the end-of-kernel DMA burst
                                nc.sync.dma_start(
                                    yp[row0:row0 + P,
                                       (fc - 1) * 512:(fc + 1) * 512],
                                    y_sb[:, (fc - 1) * 512:(fc + 1) * 512])

                # out-projection runs one group behind attention so its
                # first matmuls never wait on the freshly normalized ctx
                prev = None
                for b in range(B):
                    # ascending qc: the first group (qc=0, diagonal only) has
                    # no pipelined out-projection to interleave, so keep it
                    # small; later groups overlap prev out-proj with their
                    # scalar-paced attention
                    for qc in range(QCHUNKS):
                        d0 = 4 * qc  # first diagonal k-tile
                        ctx_tiles = []
                        for h in range(HPC):
                            ctxu_ps = cps.tile([P, QCW], F32, tag="ctxu")
                            sums_bc = sbps.tile([P, QCW], F32, tag="sumbc")
                            # full (unmasked) k-tiles, 512-wide moving dim
                            for kt in range(d0):
                                sps = spool.tile([P, QCW], F32, tag="s")
                                nc.tensor.matmul(
                                    sps[:],
                                    kT[:, h, b, kt * P:(kt + 1) * P],
                                    qT[:, h, b, qc * QCW:(qc + 1) * QCW],
                                    start=True, stop=True,
                                )
                                et = epool.tile([P, QCW], BF16, tag="e")
                                nc.scalar.activation(
                                    et[:], sps[:], AF.Exp, scale=SCALE
                                )
                                nc.tensor.matmul(
                                    ctxu_ps[:],
                                    vN[:, b * KT_PER_B + kt, h * HD:(h + 1) * HD],
                                    et[:],
                                    start=(kt == 0), stop=False,
                                )
                                # all-ones lhsT: rows of out = sums over
                                # k, i.e. reduce + broadcast in one matmul,
                                # accumulated across k-tiles in PSUM
                                nc.tensor.matmul(
                                    sums_bc[:], ones_sb[:], et[:],
                                    start=(kt == 0), stop=False,
                                )
                            # diagonal region at 256-wide granularity:
                            # q-half j2 needs k-tiles d0..d0+2*j2+1 only;
                            # tiles with a >= 2*j2 are partially masked
                            for j2 in range(2):
                                qs = qc * QCW + j2 * 256
                                for a in range(2 * j2 + 2):
                                    kt = d0 + a
                                    last = (a == 2 * j2 + 1)
                                    sps = spool.tile([P, QCW], F32, tag="s")
                                    nc.tensor.matmul(
                                        sps[:, 0:256],
                                        kT[:, h, b, kt * P:(kt + 1) * P],
                                        qT[:, h, b, qs:qs + 256],
                                        start=True, stop=True,
                                    )
                                    et = epool.tile([P, 256], BF16, tag="e256")
                                    nc.scalar.activation(
                                        et[:], sps[:, 0:256], AF.Exp, scale=SCALE
                                    )
                                    aa = a - 2 * j2
                                    if aa >= 0:  # partially masked tile
                                        nc.vector.tensor_mul(
                                            et[:], et[:],
                                            masks_sb[:, aa, 0:256]
                                        )
                                    reg = slice(j2 * 256, j2 * 256 + 256)
                                    nc.tensor.matmul(
                                        ctxu_ps[:, reg],
                                        vN[:, b * KT_PER_B + kt,
                                           h * HD:(h + 1) * HD],
                                        et[:],
                                        start=(qc == 0 and a == 0), stop=last,
                                    )
                                    nc.tensor.matmul(
                                        sums_bc[:, reg], ones_sb[:], et[:],
                                        start=(qc == 0 and a == 0), stop=last,
                                    )
                            recip_bc = smpool.tile([P, QCW], F32, tag="recipbc")
                            nc.vector.reciprocal_approx_fast(recip_bc[:], sums_bc[:])
                            ctx = ctxpool.tile([P, QCW], BF16, tag="ctx")
                            nc.vector.tensor_mul(ctx[:], ctxu_ps[:], recip_bc[:])
                            ctx_tiles.append(ctx)

                        if prev is not None:
                            outproj(*prev, nts=(0, 1, 2, 3))
                        prev = (ctx_tiles, b, qc)
                outproj(*prev, nts=(0, 1, 2, 3))

    nc.compile()
    return nc


def _host_prep(x, wq, bq, wk, bk, wv, bv, wo):
    """Build the 8 per-core input maps."""
    x = np.asarray(x, dtype=np.float32)
    xT = np.ascontiguousarray(x.reshape(BN, C).T)  # [C, BN]
    x8T = xT.astype(NP_FP8)
    xbT = xT.astype(NP_BF16)

    m = np.zeros((4, P, QCW), dtype=np.float32)
    kl = np.arange(P)[:, None]
    ql = np.arange(QCW)[None, :]
    for a in range(4):
        m[a] = (ql >= (P * a + kl)).astype(np.float32)
    m = m.astype(NP_BF16)

    wq_f = np.asarray(wq, dtype=np.float32)
    wk_f = np.asarray(wk, dtype=np.float32)
    wv_f = np.asarray(wv, dtype=np.float32)
    wo_f = np.asarray(wo, dtype=np.float32)

    bq_f = np.asarray(bq, np.float32)
    bk_f = np.asarray(bk, np.float32)
    bv_f = np.asarray(bv, np.float32)

    in_maps = []
    for c in range(NCORES):
        e0 = c * E
        wqk = np.concatenate(
            [wq_f[e0:e0 + E, :].T, wk_f[e0:e0 + E, :].T], axis=1) * WSCALE
        bqkv = np.concatenate([
            bq_f[e0:e0 + E].reshape(HPC, P),
            bk_f[e0:e0 + E].reshape(HPC, P),
            bv_f[e0:e0 + E].reshape(HPC, P)], axis=0)
        in_maps.append({
            "x8T": x8T,
            "xbT": xbT,
            "wqk8": np.ascontiguousarray(wqk).astype(NP_FP8),
            "wvT": np.ascontiguousarray(wv_f[e0:e0 + E, :].T).astype(NP_BF16),
            "woT": np.ascontiguousarray(wo_f[:, e0:e0 + E].T).astype(NP_BF16),
            "bqkv": np.ascontiguousarray(bqkv),
            "masks": m,
            "ones_d": np.ones((P, P), dtype=NP_BF16),
            "ident_d": np.eye(P, dtype=np.float32),
        })
    return in_maps


def _ensure_ntff_hook_module():
    """run_bass_kernel_spmd(trace=True) imports antenv.axon_hooks; provide a
    stub (hook=None -> tracing skipped gracefully) if the module is absent."""
    try:
        import antenv.axon_hooks  # noqa: F401
    except ImportError:
        import sys
        import types
        try:
            import antenv
        except ImportError:
            return
        mod = types.ModuleType("antenv.axon_hooks")
        state = {"hook": None}
        mod.set_axon_ntff_profile_hook = lambda h: state.__setitem__("hook", h)
        mod.get_axon_ntff_profile_hook = lambda: state["hook"]
        sys.modules["antenv.axon_hooks"] = mod
        antenv.axon_hooks = mod


def kernel(**inputs):
    _ensure_ntff_hook_module()
    if "nc" not in _CACHE:
        _CACHE["nc"] = _build()
    nc = _CACHE["nc"]

    in_maps = _host_prep(
        inputs["x"], inputs["wq"], inputs["bq"], inputs["wk"], inputs["bk"],
        inputs["wv"], inputs["bv"], inputs["wo"],
    )

    res = bass_utils.run_bass_kernel_spmd(
        nc, in_maps, core_ids=list(range(NCORES)),
        trace=bool(os.environ.get("BASS_TRACE")),
    )
    _CACHE["last_result"] = res

    y = np.zeros((BN, C), dtype=np.float64)
    for c in range(NCORES):
        y += res.results[c]["yp"].astype(np.float64)
    y += np.asarray(inputs["bo"], dtype=np.float64)
    return y.astype(np.float32).reshape(B, N, C)


# revision 45
# speedup vs baseline: 1.0022x; 1.0022x over previous
"""Trainium2 Bass kernel: causal multi-head attention (B=2, N=2048, C=2048, 16 heads).

Sharding: 16 heads split across 8 cores (2 heads/core, tensor parallel).
Each core computes q/k/v projections for its 2 heads, causal attention,
and its partial out-projection y_c = ctx_c @ wo_c.T. Host sums partials + bo.

Layout/dtype strategy (vs an all-f32r version, ~430us -> ~311us):
  q/k projections run as pure-fp8e4m3 DoubleRow matmuls (2 c-tiles per
  instruction, ~1.87x PE rate on HW). Operands are cast host-side
  (ml_dtypes) so the device error equals the host-simulated error
  (1.43e-2 < 2e-2 gate): weights are pre-scaled by 64 to clear the
  e4m3 subnormal range and descaled in the PSUM->SBUF activation.
  v projection, scores, AV, row sums and out-proj run in bf16 (same PE
  rate as f32r, adds only ~3.5e-3 err). Output is bf16 (halves output
  DMA); host sums partials in f64.

Per-core layout:
  qT/kT: [head_dim(128) partitions, tokens] bf16  (from lhsT=w^T, rhs=x^T)
  vT is PE-transposed (f32) to V natural [tok, d], stored bf16
  S^T[k, q] = K^T.T @ Q^T tiles (contraction over head_dim), bf16 operands
  E^T = exp(scale * S^T) bf16 (no max subtraction -- scores are ~N(0,1/9))
  ctx^T[d, q] = V.T @ E^T (bf16); row sums via all-ones-lhsT matmuls
  (reduce+broadcast in PSUM), normalized on DVE with fast reciprocal
  y[tok, f] = ctx^T.T @ wo^T (bf16 operands, bf16 output)

Schedule notes (what the ~88% PE occupancy comes from):
  - Phase 1 runs the fp8 q/k sweep before the bf16 v sweep each chunk,
    with DMA issue order matched (few, large transfers: the sync engine
    issues DMAs at ~650ns each, pacing the kernel start).
  - Causality at tile granularity (k-tile <= q-tile), and the 4
    diagonal k-tiles are computed at 256-wide granularity: q-half j2
    needs k-tiles d0..d0+2*j2+1 only (-25% diagonal PE work); partial
    tiles are masked by 0/1 mask multiply after exp.
  - The attention inner loop is paced by the scalar-engine exp
    (~686ns/tile vs ~645ns of PE work), so each group's out-projection
    is software-pipelined one group behind and its PE-heavy matmuls
    fill the scalar-paced slack; ascending qc keeps the first
    (unfillable) group small.
  - Output y streams out per-2-fc-block DMAs to avoid a tail burst.
"""

import os
import numpy as np
import ml_dtypes

import concourse.bass as bass
import concourse.tile as tile
from concourse import bacc, mybir
from concourse import bass_utils

F32 = mybir.dt.float32
BF16 = mybir.dt.bfloat16
FP8 = mybir.dt.float8e4
AF = mybir.ActivationFunctionType
DR = mybir.MatmulPerfMode.DoubleRow

NP_BF16 = ml_dtypes.bfloat16
NP_FP8 = ml_dtypes.float8_e4m3

# problem dims (hardcoded per contract)
B = 2
N = 2048
C = 2048
HEADS = 16
HD = 128          # head dim
NCORES = 8
HPC = HEADS // NCORES  # heads per core = 2
E = HPC * HD      # per-core projection width = 256
BN = B * N        # 4096
P = 128
CT = C // P       # 16 contraction tiles
NCH = 512         # n-chunk width for projections
NCHUNKS = BN // NCH   # 8
QCW = 512         # q-chunk width in attention
QCHUNKS = N // QCW    # 4 per batch
KT_PER_B = N // P     # 16 k-tiles per batch
TOK_TILES = BN // P   # 32
SCALE = float(HD) ** -0.5
WSCALE = 64.0     # host pre-scale on wq/wk to clear e4m3 subnormals
XQ = 4            # x streamed in quarters of 4 c-tiles

_CACHE = {}


def _build():
    nc = bacc.Bacc(
        "TRN2",
        target_bir_lowering=False,
        debug=False,
        enable_asserts=False,
        num_devices=NCORES,
    )

    x8T = nc.dram_tensor("x8T", [C, BN], FP8, kind="ExternalInput").ap()
    xbT = nc.dram_tensor("xbT", [C, BN], BF16, kind="ExternalInput").ap()
    wqk8 = nc.dram_tensor("wqk8", [C, 2 * E], FP8, kind="ExternalInput").ap()
    wvT = nc.dram_tensor("wvT", [C, E], BF16, kind="ExternalInput").ap()
    woT = nc.dram_tensor("woT", [E, C], BF16, kind="ExternalInput").ap()
    bqkv = nc.dram_tensor("bqkv", [3 * HPC, P], F32, kind="ExternalInput").ap()
    masks = nc.dram_tensor("masks", [4, P, QCW], BF16, kind="ExternalInput").ap()
    ones_d = nc.dram_tensor("ones_d", [P, P], BF16, kind="ExternalInput").ap()
    ident_d = nc.dram_tensor("ident_d", [P, P], F32, kind="ExternalInput").ap()
    yp = nc.dram_tensor("yp", [BN, C], BF16, kind="ExternalOutput").ap()

    with tile.TileContext(nc) as tc:
        with tc.tile_pool(name="persist", bufs=1) as persist:
            # persistent per-core activations
            qT = persist.tile([P, HPC, B, N], BF16, tag="qT")
            kT = persist.tile([P, HPC, B, N], BF16, tag="kT")
            vN = persist.tile([P, TOK_TILES, E], BF16, tag="vN")
            masks_sb = persist.tile([P, 4, QCW], BF16, tag="masks")
            ones_sb = persist.tile([P, P], BF16, tag="ones")
            ident_sb = persist.tile([P, P], F32, tag="ident")

            # ---------------- Phase 1: projections ----------------
            with tc.tile_pool(name="p1w", bufs=1) as wpool, \
                 tc.tile_pool(name="p1x8", bufs=8) as x8pool, \
                 tc.tile_pool(name="p1xb", bufs=8) as xbpool, \
                 tc.tile_pool(name="p1vt", bufs=2) as vtpool, \
                 tc.tile_pool(name="p1_ps", bufs=6, space="PSUM") as pps, \
                 tc.tile_pool(name="p1t_ps", bufs=2, space="PSUM") as tps:
                wqk_sb = wpool.tile([P, CT, 2 * E], FP8, tag="wqk")
                wv_sb = wpool.tile([P, CT, E], BF16, tag="wv")
                b_sb = wpool.tile([P, 3 * HPC], F32, tag="bqkv")

                # DMA priority, matched to the new consumption order:
                # q/k weights + chunk-0 x8 pieces first (the qk DR sweep
                # starts within ~2us), then wv + xb pieces (v sweep runs
                # while they land), then phase-2 constants. Few, large
                # transfers: the sync engine issues DMAs at ~650ns each,
                # which paces the kernel start.
                nc.sync.dma_start(b_sb[:], bqkv.rearrange("h p -> p h"))
                wqk8r = wqk8.rearrange("(t p) e -> p t e", p=P)
                wvTr = wvT.rearrange("(t p) e -> p t e", p=P)
                x8Tr = x8T.rearrange("(t p) n -> p t n", p=P)
                xbTr = xbT.rearrange("(t p) n -> p t n", p=P)
                x8h0 = []
                xbh0 = []
                for piece in range(XQ):
                    sl = slice(piece * 4, (piece + 1) * 4)
                    nc.sync.dma_start(wqk_sb[:, sl, :], wqk8r[:, sl, :])
                    x8c = x8pool.tile([P, 4, NCH], FP8, tag="x8c")
                    nc.sync.dma_start(x8c[:], x8Tr[:, sl, 0:NCH])
                    x8h0.append(x8c)
                for piece in range(XQ):
                    sl = slice(piece * 4, (piece + 1) * 4)
                    nc.sync.dma_start(wv_sb[:, sl, :], wvTr[:, sl, :])
                    xbc = xbpool.tile([P, 4, NCH], BF16, tag="xbc")
                    nc.sync.dma_start(xbc[:], xbTr[:, sl, 0:NCH])
                    xbh0.append(xbc)
                nc.sync.dma_start(masks_sb[:], masks.rearrange("a p n -> p a n"))
                nc.sync.dma_start(ones_sb[:], ones_d)
                nc.sync.dma_start(ident_sb[:], ident_d)

                # PE warm-up: the tensor engine reaches full clock only
                # after ~3us of continuous work, and the first real matmul
                # can't start until the initial DMAs land (~11us). Matmuls
                # on zeroed scratch (results never read) keep the PE
                # clocked through the wait so chunk 0 runs at full rate.
                wwarm = wpool.tile([P, P], BF16, tag="wwarm")
                xwarm = wpool.tile([P, NCH], BF16, tag="xwarm")
                nc.any.memset(wwarm[:], 0.0)
                nc.any.memset(xwarm[:], 0.0)
                for i in range(22):
                    pw = pps.tile([P, NCH], F32, tag="pacc",
                                  name=f"pwarm_{i}")
                    nc.tensor.matmul(pw[:], wwarm[:], xwarm[:],
                                     start=True, stop=True)

                for ch in range(NCHUNKS):
                    b = ch // (N // NCH)
                    nn0 = (ch % (N // NCH)) * NCH  # within-batch token offset
                    n0 = ch * NCH                  # global token offset
                    if ch == 0:
                        x8h, xbh = x8h0, xbh0
                    else:
                        x8h, xbh = [], []
                        for piece in range(XQ):
                            x8c = x8pool.tile([P, 4, NCH], FP8, tag="x8c")
                            nc.sync.dma_start(
                                x8c[:], x8Tr[:, piece * 4:(piece + 1) * 4,
                                             n0:n0 + NCH])
                            x8h.append(x8c)
                        for piece in range(XQ):
                            xbc = xbpool.tile([P, 4, NCH], BF16, tag="xbc")
                            nc.sync.dma_start(
                                xbc[:], xbTr[:, piece * 4:(piece + 1) * 4,
                                             n0:n0 + NCH])
                            xbh.append(xbc)

                    # accumulators: q,k (fp8 DoubleRow) + v (bf16) x 2 heads
                    accq = [pps.tile([P, NCH], F32, tag="pacc",
                                     name=f"paccq_{ch}_{h}") for h in range(HPC)]
                    acck = [pps.tile([P, NCH], F32, tag="pacc",
                                     name=f"pacck_{ch}_{h}") for h in range(HPC)]
                    accv = [pps.tile([P, NCH], F32, tag="pacc",
                                     name=f"paccv_{ch}_{h}") for h in range(HPC)]
                    # qk DoubleRow sweep first (x8 pieces arrive first)
                    for cp in range(CT // 2):
                        x8q = x8h[cp // 2][:, (cp % 2) * 2:(cp % 2) * 2 + 2, :]
                        st = (cp == 0)
                        sp = (cp == CT // 2 - 1)
                        for h in range(HPC):
                            nc.tensor.matmul(
                                accq[h][:],
                                wqk_sb[:, 2 * cp:2 * cp + 2,
                                       h * HD:(h + 1) * HD],
                                x8q, start=st, stop=sp, perf_mode=DR)
                            nc.tensor.matmul(
                                acck[h][:],
                                wqk_sb[:, 2 * cp:2 * cp + 2,
                                       E + h * HD:E + (h + 1) * HD],
                                x8q, start=st, stop=sp, perf_mode=DR)
                    for h in range(HPC):
                        nc.scalar.activation(
                            qT[:, h, b, nn0:nn0 + NCH], accq[h][:],
                            AF.Identity, bias=b_sb[:, h:h + 1],
                            scale=1.0 / WSCALE)
                        nc.scalar.activation(
                            kT[:, h, b, nn0:nn0 + NCH], acck[h][:],
                            AF.Identity, bias=b_sb[:, HPC + h:HPC + h + 1],
                            scale=1.0 / WSCALE)
                    # v sweep (bf16 plain matmuls; xb pieces land during qk)
                    for cp in range(CT // 2):
                        for h in range(HPC):
                            for sub in range(2):
                                ct = 2 * cp + sub
                                nc.tensor.matmul(
                                    accv[h][:],
                                    wv_sb[:, ct, h * HD:(h + 1) * HD],
                                    xbh[ct // 4][:, ct % 4, :],
                                    start=(ct == 0), stop=(ct == CT - 1))
                    for h in range(HPC):
                        # v with bias (f32), then PE-transpose to V natural
                        vt = vtpool.tile([P, NCH], F32, tag="vt")
                        nc.scalar.activation(
                            vt[:], accv[h][:],
                            AF.Identity,
                            bias=b_sb[:, 2 * HPC + h:2 * HPC + h + 1],
                            scale=1.0)
                        for ts in range(NCH // P):
                            tp = tps.tile([P, P], F32, tag="tp")
                            nc.tensor.transpose(
                                tp[:], vt[:, ts * P:(ts + 1) * P], ident_sb[:])
                            nc.vector.tensor_copy(
                                vN[:, ch * (NCH // P) + ts,
                                   h * HD:(h + 1) * HD],
                                tp[:])

            # ---------------- Phase 2: attention + out-proj ----------------
            with tc.tile_pool(name="p2const", bufs=1) as cpool, \
                 tc.tile_pool(name="p2e", bufs=8) as epool, \
                 tc.tile_pool(name="p2ctx", bufs=6) as ctxpool, \
                 tc.tile_pool(name="p2sm", bufs=3) as smpool, \
                 tc.tile_pool(name="p2y", bufs=3) as ysbpool, \
                 tc.tile_pool(name="p2s_ps", bufs=2, space="PSUM") as spool, \
                 tc.tile_pool(name="p2c_ps", bufs=2, space="PSUM") as cps, \
                 tc.tile_pool(name="p2sb_ps", bufs=2, space="PSUM") as sbps, \
                 tc.tile_pool(name="p2y_ps", bufs=2, space="PSUM") as yps:
                wo_sb = cpool.tile([P, HPC, C], BF16, tag="wo")
                nc.sync.dma_start(wo_sb[:], woT.rearrange("(h p) f -> p h f", p=P))

                def outproj(ctx_tiles, b, qc, nts):
                    for nt in nts:
                        y_sb = ysbpool.tile([P, C], BF16, tag="ysb")
                        row0 = b * N + qc * QCW + nt * P
                        for fc in range(C // 512):
                            y_ps = yps.tile([P, 512], F32, tag="yps")
                            for h in range(HPC):
                                nc.tensor.matmul(
                                    y_ps[:],
                                    ctx_tiles[h][:, nt * P:(nt + 1) * P],
                                    wo_sb[:, h, fc * 512:(fc + 1) * 512],
                                    start=(h == 0), stop=(h == HPC - 1),
                                )
                            if fc % 2 == 0:
                                nc.vector.tensor_copy(
                                    y_sb[:, fc * 512:(fc + 1) * 512],
                                    y_ps[:])
                            else:
                                nc.scalar.copy(
                                    y_sb[:, fc * 512:(fc + 1) * 512],
                                    y_ps[:])
                                # drain output as it is produced: halves
                                # the end-of-kernel DMA burst
                                nc.sync.dma_start(
                                    yp[row0:row0 + P,
                                       (fc - 1) * 512:(fc + 1) * 512],
                                    y_sb[:, (fc - 1) * 512:(fc + 1) * 512])

                # out-projection runs one group behind attention so its
                # first matmuls never wait on the freshly normalized ctx
                prev = None
                for b in range(B):
                    # ascending qc: the first group (qc=0, diagonal only) has
                    # no pipelined out-projection to interleave, so keep it
                    # small; later groups overlap prev out-proj with their
                    # scalar-paced attention
                    for qc in range(QCHUNKS):
                        d0 = 4 * qc  # first diagonal k-tile
                        ctx_tiles = []
                        for h in range(HPC):
                            ctxu_ps = cps.tile([P, QCW], F32, tag="ctxu")
                            sums_bc = sbps.tile([P, QCW], F32, tag="sumbc")
                            # full (unmasked) k-tiles, 512-wide moving dim
                            for kt in range(d0):
                                sps = spool.tile([P, QCW], F32, tag="s")
                                nc.tensor.matmul(
                                    sps[:],
                                    kT[:, h, b, kt * P:(kt + 1) * P],
                                    qT[:, h, b, qc * QCW:(qc + 1) * QCW],
                                    start=True, stop=True,
                                )
                                et = epool.tile([P, QCW], BF16, tag="e")
                                nc.scalar.activation(
                                    et[:], sps[:], AF.Exp, scale=SCALE
                                )
                                nc.tensor.matmul(
                                    ctxu_ps[:],
                                    vN[:, b * KT_PER_B + kt, h * HD:(h + 1) * HD],
                                    et[:],
                                    start=(kt == 0), stop=False,
                                )
                                # all-ones lhsT: rows of out = sums over
                                # k, i.e. reduce + broadcast in one matmul,
                                # accumulated across k-tiles in PSUM
                                nc.tensor.matmul(
                                    sums_bc[:], ones_sb[:], et[:],
                                    start=(kt == 0), stop=False,
                                )
                            # diagonal region at 256-wide granularity:
                            # q-half j2 needs k-tiles d0..d0+2*j2+1 only;
                            # tiles with a >= 2*j2 are partially masked
                            for j2 in range(2):
                                qs = qc * QCW + j2 * 256
                                for a in range(2 * j2 + 2):
                                    kt = d0 + a
                                    last = (a == 2 * j2 + 1)
                                    sps = spool.tile([P, QCW], F32, tag="s")
                                    nc.tensor.matmul(
                                        sps[:, 0:256],
                                        kT[:, h, b, kt * P:(kt + 1) * P],
                                        qT[:, h, b, qs:qs + 256],
                                        start=True, stop=True,
                                    )
                                    et = epool.tile([P, 256], BF16, tag="e256")
                                    nc.scalar.activation(
                                        et[:], sps[:, 0:256], AF.Exp, scale=SCALE
                                    )
                                    aa = a - 2 * j2
                                    if aa >= 0:  # partially masked tile
                                        nc.vector.tensor_mul(
                                            et[:], et[:],
                                            masks_sb[:, aa, 0:256]
                                        )
                                    reg = slice(j2 * 256, j2 * 256 + 256)
                                    nc.tensor.matmul(
                                        ctxu_ps[:, reg],
                                        vN[:, b * KT_PER_B + kt,
                                           h * HD:(h + 1) * HD],
                                        et[:],
                                        start=(qc == 0 and a == 0), stop=last,
                                    )
                                    nc.tensor.matmul(
                                        sums_bc[:, reg], ones_sb[:], et[:],
                                        start=(qc == 0 and a == 0), stop=last,
                                    )
                            recip_bc = smpool.tile([P, QCW], F32, tag="recipbc")
                            nc.vector.reciprocal_approx_fast(recip_bc[:], sums_bc[:])
                            ctx = ctxpool.tile([P, QCW], BF16, tag="ctx")
                            nc.vector.tensor_mul(ctx[:], ctxu_ps[:], recip_bc[:])
                            ctx_tiles.append(ctx)

                        if prev is not None:
                            outproj(*prev, nts=(0, 1, 2, 3))
                        prev = (ctx_tiles, b, qc)
                outproj(*prev, nts=(0, 1, 2, 3))

    nc.compile()
    return nc


def _host_prep(x, wq, bq, wk, bk, wv, bv, wo):
    """Build the 8 per-core input maps."""
    x = np.asarray(x, dtype=np.float32)
    xT = np.ascontiguousarray(x.reshape(BN, C).T)  # [C, BN]
    x8T = xT.astype(NP_FP8)
    xbT = xT.astype(NP_BF16)

    m = np.zeros((4, P, QCW), dtype=np.float32)
    kl = np.arange(P)[:, None]
    ql = np.arange(QCW)[None, :]
    for a in range(4):
        m[a] = (ql >= (P * a + kl)).astype(np.float32)
    m = m.astype(NP_BF16)

    wq_f = np.asarray(wq, dtype=np.float32)
    wk_f = np.asarray(wk, dtype=np.float32)
    wv_f = np.asarray(wv, dtype=np.float32)
    wo_f = np.asarray(wo, dtype=np.float32)

    bq_f = np.asarray(bq, np.float32)
    bk_f = np.asarray(bk, np.float32)
    bv_f = np.asarray(bv, np.float32)

    in_maps = []
    for c in range(NCORES):
        e0 = c * E
        wqk = np.concatenate(
            [wq_f[e0:e0 + E, :].T, wk_f[e0:e0 + E, :].T], axis=1) * WSCALE
        bqkv = np.concatenate([
            bq_f[e0:e0 + E].reshape(HPC, P),
            bk_f[e0:e0 + E].reshape(HPC, P),
            bv_f[e0:e0 + E].reshape(HPC, P)], axis=0)
        in_maps.append({
            "x8T": x8T,
            "xbT": xbT,
            "wqk8": np.ascontiguousarray(wqk).astype(NP_FP8),
            "wvT": np.ascontiguousarray(wv_f[e0:e0 + E, :].T).astype(NP_BF16),
            "woT": np.ascontiguousarray(wo_f[:, e0:e0 + E].T).astype(NP_BF16),
            "bqkv": np.ascontiguousarray(bqkv),
            "masks": m,
            "ones_d": np.ones((P, P), dtype=NP_BF16),
            "ident_d": np.eye(P, dtype=np.float32),
        })
    return in_maps


def _ensure_ntff_hook_module():
    """run_bass_kernel_spmd(trace=True) imports antenv.axon_hooks; provide a
    stub (hook=None -> tracing skipped gracefully) if the module is absent."""
    try:
        import antenv.axon_hooks  # noqa: F401
    except ImportError:
        import sys
        import types
        try:
            import antenv
        except ImportError:
            return
        mod = types.ModuleType("antenv.axon_hooks")
        state = {"hook": None}
        mod.set_axon_ntff_profile_hook = lambda h: state.__setitem__("hook", h)
        mod.get_axon_ntff_profile_hook = lambda: state["hook"]
        sys.modules["antenv.axon_hooks"] = mod
        antenv.axon_hooks = mod


def kernel(**inputs):
    _ensure_ntff_hook_module()
    if "nc" not in _CACHE:
        _CACHE["nc"] = _build()
    nc = _CACHE["nc"]

    in_maps = _host_prep(
        inputs["x"], inputs["wq"], inputs["bq"], inputs["wk"], inputs["bk"],
        inputs["wv"], inputs["bv"], inputs["wo"],
    )

    res = bass_utils.run_bass_kernel_spmd(
        nc, in_maps, core_ids=list(range(NCORES)),
        trace=bool(os.environ.get("BASS_TRACE")),
    )
    _CACHE["last_result"] = res

    y = np.zeros((BN, C), dtype=np.float64)
    for c in range(NCORES):
        y += res.results[c]["yp"].astype(np.float64)
    y += np.asarray(inputs["bo"], dtype=np.float64)
    return y.astype(np.float32).reshape(B, N, C)


# revision 46
# speedup vs baseline: 1.0060x; 1.0038x over previous
"""Trainium2 Bass kernel: causal multi-head attention (B=2, N=2048, C=2048, 16 heads).

Sharding: 16 heads split across 8 cores (2 heads/core, tensor parallel).
Each core computes q/k/v projections for its 2 heads, causal attention,
and its partial out-projection y_c = ctx_c @ wo_c.T. Host sums partials + bo.

Layout/dtype strategy (vs an all-f32r version, ~430us -> ~311us):
  q/k projections run as pure-fp8e4m3 DoubleRow matmuls (2 c-tiles per
  instruction, ~1.87x PE rate on HW). Operands are cast host-side
  (ml_dtypes) so the device error equals the host-simulated error
  (1.43e-2 < 2e-2 gate): weights are pre-scaled by 64 to clear the
  e4m3 subnormal range and descaled in the PSUM->SBUF activation.
  v projection, scores, AV, row sums and out-proj run in bf16 (same PE
  rate as f32r, adds only ~3.5e-3 err). Output is bf16 (halves output
  DMA); host sums partials in f64.

Per-core layout:
  qT/kT: [head_dim(128) partitions, tokens] bf16  (from lhsT=w^T, rhs=x^T)
  vT is PE-transposed (f32) to V natural [tok, d], stored bf16
  S^T[k, q] = K^T.T @ Q^T tiles (contraction over head_dim), bf16 operands
  E^T = exp(scale * S^T) bf16 (no max subtraction -- scores are ~N(0,1/9))
  ctx^T[d, q] = V.T @ E^T (bf16); row sums via all-ones-lhsT matmuls
  (reduce+broadcast in PSUM), normalized on DVE with fast reciprocal
  y[tok, f] = ctx^T.T @ wo^T (bf16 operands, bf16 output)

Schedule notes (what the ~88% PE occupancy comes from):
  - Phase 1 runs the fp8 q/k sweep before the bf16 v sweep each chunk,
    with DMA issue order matched (few, large transfers: the sync engine
    issues DMAs at ~650ns each, pacing the kernel start).
  - Causality at tile granularity (k-tile <= q-tile), and the 4
    diagonal k-tiles are computed at 256-wide granularity: q-half j2
    needs k-tiles d0..d0+2*j2+1 only (-25% diagonal PE work); partial
    tiles are masked by 0/1 mask multiply after exp.
  - The attention inner loop is paced by the scalar-engine exp
    (~686ns/tile vs ~645ns of PE work), so each group's out-projection
    is software-pipelined one group behind and its PE-heavy matmuls
    fill the scalar-paced slack; ascending qc keeps the first
    (unfillable) group small.
  - Output y streams out per-2-fc-block DMAs to avoid a tail burst.
"""

import os
import numpy as np
import ml_dtypes

import concourse.bass as bass
import concourse.tile as tile
from concourse import bacc, mybir
from concourse import bass_utils

F32 = mybir.dt.float32
BF16 = mybir.dt.bfloat16
FP8 = mybir.dt.float8e4
AF = mybir.ActivationFunctionType
DR = mybir.MatmulPerfMode.DoubleRow

NP_BF16 = ml_dtypes.bfloat16
NP_FP8 = ml_dtypes.float8_e4m3

# problem dims (hardcoded per contract)
B = 2
N = 2048
C = 2048
HEADS = 16
HD = 128          # head dim
NCORES = 8
HPC = HEADS // NCORES  # heads per core = 2
E = HPC * HD      # per-core projection width = 256
BN = B * N        # 4096
P = 128
CT = C // P       # 16 contraction tiles
NCH = 512         # n-chunk width for projections
NCHUNKS = BN // NCH   # 8
QCW = 512         # q-chunk width in attention
QCHUNKS = N // QCW    # 4 per batch
KT_PER_B = N // P     # 16 k-tiles per batch
TOK_TILES = BN // P   # 32
SCALE = float(HD) ** -0.5
WSCALE = 64.0     # host pre-scale on wq/wk to clear e4m3 subnormals
XQ = 4            # x streamed in quarters of 4 c-tiles

_CACHE = {}


def _build():
    nc = bacc.Bacc(
        "TRN2",
        target_bir_lowering=False,
        debug=False,
        enable_asserts=False,
        num_devices=NCORES,
    )

    x8T = nc.dram_tensor("x8T", [C, BN], FP8, kind="ExternalInput").ap()
    xbT = nc.dram_tensor("xbT", [C, BN], BF16, kind="ExternalInput").ap()
    wqk8 = nc.dram_tensor("wqk8", [C, 2 * E], FP8, kind="ExternalInput").ap()
    wvT = nc.dram_tensor("wvT", [C, E], BF16, kind="ExternalInput").ap()
    woT = nc.dram_tensor("woT", [E, C], BF16, kind="ExternalInput").ap()
    bqkv = nc.dram_tensor("bqkv", [3 * HPC, P], F32, kind="ExternalInput").ap()
    masks = nc.dram_tensor("masks", [4, P, QCW], BF16, kind="ExternalInput").ap()
    ones_d = nc.dram_tensor("ones_d", [P, P], BF16, kind="ExternalInput").ap()
    ident_d = nc.dram_tensor("ident_d", [P, P], F32, kind="ExternalInput").ap()
    yp = nc.dram_tensor("yp", [BN, C], BF16, kind="ExternalOutput").ap()

    with tile.TileContext(nc) as tc:
        with tc.tile_pool(name="persist", bufs=1) as persist:
            # persistent per-core activations
            qT = persist.tile([P, HPC, B, N], BF16, tag="qT")
            kT = persist.tile([P, HPC, B, N], BF16, tag="kT")
            vN = persist.tile([P, TOK_TILES, E], BF16, tag="vN")
            masks_sb = persist.tile([P, 4, QCW], BF16, tag="masks")
            ones_sb = persist.tile([P, P], BF16, tag="ones")
            ident_sb = persist.tile([P, P], F32, tag="ident")

            # ---------------- Phase 1: projections ----------------
            with tc.tile_pool(name="p1w", bufs=1) as wpool, \
                 tc.tile_pool(name="p1x8", bufs=8) as x8pool, \
                 tc.tile_pool(name="p1xb", bufs=8) as xbpool, \
                 tc.tile_pool(name="p1vt", bufs=2) as vtpool, \
                 tc.tile_pool(name="p1_ps", bufs=6, space="PSUM") as pps, \
                 tc.tile_pool(name="p1t_ps", bufs=2, space="PSUM") as tps:
                wqk_sb = wpool.tile([P, CT, 2 * E], FP8, tag="wqk")
                wv_sb = wpool.tile([P, CT, E], BF16, tag="wv")
                b_sb = wpool.tile([P, 3 * HPC], F32, tag="bqkv")

                # DMA priority, matched to the new consumption order:
                # q/k weights + chunk-0 x8 pieces first (the qk DR sweep
                # starts within ~2us), then wv + xb pieces (v sweep runs
                # while they land), then phase-2 constants. Few, large
                # transfers: the sync engine issues DMAs at ~650ns each,
                # which paces the kernel start.
                nc.sync.dma_start(b_sb[:], bqkv.rearrange("h p -> p h"))
                wqk8r = wqk8.rearrange("(t p) e -> p t e", p=P)
                wvTr = wvT.rearrange("(t p) e -> p t e", p=P)
                x8Tr = x8T.rearrange("(t p) n -> p t n", p=P)
                xbTr = xbT.rearrange("(t p) n -> p t n", p=P)
                x8h0 = []
                xbh0 = []
                for piece in range(XQ):
                    sl = slice(piece * 4, (piece + 1) * 4)
                    nc.sync.dma_start(wqk_sb[:, sl, :], wqk8r[:, sl, :])
                    x8c = x8pool.tile([P, 4, NCH], FP8, tag="x8c")
                    nc.sync.dma_start(x8c[:], x8Tr[:, sl, 0:NCH])
                    x8h0.append(x8c)
                for piece in range(XQ):
                    sl = slice(piece * 4, (piece + 1) * 4)
                    nc.sync.dma_start(wv_sb[:, sl, :], wvTr[:, sl, :])
                    xbc = xbpool.tile([P, 4, NCH], BF16, tag="xbc")
                    nc.sync.dma_start(xbc[:], xbTr[:, sl, 0:NCH])
                    xbh0.append(xbc)
                nc.sync.dma_start(masks_sb[:], masks.rearrange("a p n -> p a n"))
                nc.sync.dma_start(ones_sb[:], ones_d)
                nc.sync.dma_start(ident_sb[:], ident_d)

                for ch in range(NCHUNKS):
                    b = ch // (N // NCH)
                    nn0 = (ch % (N // NCH)) * NCH  # within-batch token offset
                    n0 = ch * NCH                  # global token offset
                    if ch == 0:
                        x8h, xbh = x8h0, xbh0
                    else:
                        x8h, xbh = [], []
                        for piece in range(XQ):
                            x8c = x8pool.tile([P, 4, NCH], FP8, tag="x8c")
                            nc.sync.dma_start(
                                x8c[:], x8Tr[:, piece * 4:(piece + 1) * 4,
                                             n0:n0 + NCH])
                            x8h.append(x8c)
                        for piece in range(XQ):
                            xbc = xbpool.tile([P, 4, NCH], BF16, tag="xbc")
                            nc.sync.dma_start(
                                xbc[:], xbTr[:, piece * 4:(piece + 1) * 4,
                                             n0:n0 + NCH])
                            xbh.append(xbc)

                    # accumulators: q,k (fp8 DoubleRow) + v (bf16) x 2 heads
                    accq = [pps.tile([P, NCH], F32, tag="pacc",
                                     name=f"paccq_{ch}_{h}") for h in range(HPC)]
                    acck = [pps.tile([P, NCH], F32, tag="pacc",
                                     name=f"pacck_{ch}_{h}") for h in range(HPC)]
                    accv = [pps.tile([P, NCH], F32, tag="pacc",
                                     name=f"paccv_{ch}_{h}") for h in range(HPC)]
                    # qk DoubleRow sweep first (x8 pieces arrive first)
                    for cp in range(CT // 2):
                        x8q = x8h[cp // 2][:, (cp % 2) * 2:(cp % 2) * 2 + 2, :]
                        st = (cp == 0)
                        sp = (cp == CT // 2 - 1)
                        for h in range(HPC):
                            nc.tensor.matmul(
                                accq[h][:],
                                wqk_sb[:, 2 * cp:2 * cp + 2,
                                       h * HD:(h + 1) * HD],
                                x8q, start=st, stop=sp, perf_mode=DR)
                            nc.tensor.matmul(
                                acck[h][:],
                                wqk_sb[:, 2 * cp:2 * cp + 2,
                                       E + h * HD:E + (h + 1) * HD],
                                x8q, start=st, stop=sp, perf_mode=DR)
                    for h in range(HPC):
                        nc.scalar.activation(
                            qT[:, h, b, nn0:nn0 + NCH], accq[h][:],
                            AF.Identity, bias=b_sb[:, h:h + 1],
                            scale=1.0 / WSCALE)
                        nc.scalar.activation(
                            kT[:, h, b, nn0:nn0 + NCH], acck[h][:],
                            AF.Identity, bias=b_sb[:, HPC + h:HPC + h + 1],
                            scale=1.0 / WSCALE)
                    # v sweep (bf16 plain matmuls; xb pieces land during qk)
                    for cp in range(CT // 2):
                        for h in range(HPC):
                            for sub in range(2):
                                ct = 2 * cp + sub
                                nc.tensor.matmul(
                                    accv[h][:],
                                    wv_sb[:, ct, h * HD:(h + 1) * HD],
                                    xbh[ct // 4][:, ct % 4, :],
                                    start=(ct == 0), stop=(ct == CT - 1))
                    for h in range(HPC):
                        # v with bias (f32), then PE-transpose to V natural
                        vt = vtpool.tile([P, NCH], F32, tag="vt")
                        nc.scalar.activation(
                            vt[:], accv[h][:],
                            AF.Identity,
                            bias=b_sb[:, 2 * HPC + h:2 * HPC + h + 1],
                            scale=1.0)
                        for ts in range(NCH // P):
                            tp = tps.tile([P, P], F32, tag="tp")
                            nc.tensor.transpose(
                                tp[:], vt[:, ts * P:(ts + 1) * P], ident_sb[:])
                            nc.vector.tensor_copy(
                                vN[:, ch * (NCH // P) + ts,
                                   h * HD:(h + 1) * HD],
                                tp[:])

            # ---------------- Phase 2: attention + out-proj ----------------
            with tc.tile_pool(name="p2const", bufs=1) as cpool, \
                 tc.tile_pool(name="p2e", bufs=8) as epool, \
                 tc.tile_pool(name="p2ctx", bufs=6) as ctxpool, \
                 tc.tile_pool(name="p2sm", bufs=3) as smpool, \
                 tc.tile_pool(name="p2y", bufs=3) as ysbpool, \
                 tc.tile_pool(name="p2s_ps", bufs=2, space="PSUM") as spool, \
                 tc.tile_pool(name="p2c_ps", bufs=2, space="PSUM") as cps, \
                 tc.tile_pool(name="p2sb_ps", bufs=2, space="PSUM") as sbps, \
                 tc.tile_pool(name="p2y_ps", bufs=2, space="PSUM") as yps:
                wo_sb = cpool.tile([P, HPC, C], BF16, tag="wo")
                nc.sync.dma_start(wo_sb[:], woT.rearrange("(h p) f -> p h f", p=P))

                def outproj(ctx_tiles, b, qc, nts):
                    for nt in nts:
                        y_sb = ysbpool.tile([P, C], BF16, tag="ysb")
                        row0 = b * N + qc * QCW + nt * P
                        for fc in range(C // 512):
                            y_ps = yps.tile([P, 512], F32, tag="yps")
                            for h in range(HPC):
                                nc.tensor.matmul(
                                    y_ps[:],
                                    ctx_tiles[h][:, nt * P:(nt + 1) * P],
                                    wo_sb[:, h, fc * 512:(fc + 1) * 512],
                                    start=(h == 0), stop=(h == HPC - 1),
                                )
                            if fc % 2 == 0:
                                nc.vector.tensor_copy(
                                    y_sb[:, fc * 512:(fc + 1) * 512],
                                    y_ps[:])
                            else:
                                nc.scalar.copy(
                                    y_sb[:, fc * 512:(fc + 1) * 512],
                                    y_ps[:])
                                # drain output as it is produced: halves
                                # the end-of-kernel DMA burst
                                nc.sync.dma_start(
                                    yp[row0:row0 + P,
                                       (fc - 1) * 512:(fc + 1) * 512],
                                    y_sb[:, (fc - 1) * 512:(fc + 1) * 512])

                # out-projection runs one group behind attention so its
                # first matmuls never wait on the freshly normalized ctx
                prev = None
                for b in range(B):
                    # ascending qc: the first group (qc=0, diagonal only) has
                    # no pipelined out-projection to interleave, so keep it
                    # small; later groups overlap prev out-proj with their
                    # scalar-paced attention
                    for qc in range(QCHUNKS):
                        d0 = 4 * qc  # first diagonal k-tile
                        ctx_tiles = []
                        for h in range(HPC):
                            ctxu_ps = cps.tile([P, QCW], F32, tag="ctxu")
                            sums_bc = sbps.tile([P, QCW], F32, tag="sumbc")
                            # full (unmasked) k-tiles, 512-wide moving dim
                            for kt in range(d0):
                                sps = spool.tile([P, QCW], F32, tag="s")
                                nc.tensor.matmul(
                                    sps[:],
                                    kT[:, h, b, kt * P:(kt + 1) * P],
                                    qT[:, h, b, qc * QCW:(qc + 1) * QCW],
                                    start=True, stop=True,
                                )
                                et = epool.tile([P, QCW], BF16, tag="e")
                                nc.scalar.activation(
                                    et[:], sps[:], AF.Exp, scale=SCALE
                                )
                                nc.tensor.matmul(
                                    ctxu_ps[:],
                                    vN[:, b * KT_PER_B + kt, h * HD:(h + 1) * HD],
                                    et[:],
                                    start=(kt == 0), stop=False,
                                )
                                # all-ones lhsT: rows of out = sums over
                                # k, i.e. reduce + broadcast in one matmul,
                                # accumulated across k-tiles in PSUM
                                nc.tensor.matmul(
                                    sums_bc[:], ones_sb[:], et[:],
                                    start=(kt == 0), stop=False,
                                )
                            # diagonal region at 256-wide granularity:
                            # q-half j2 needs k-tiles d0..d0+2*j2+1 only;
                            # tiles with a >= 2*j2 are partially masked
                            for j2 in range(2):
                                qs = qc * QCW + j2 * 256
                                for a in range(2 * j2 + 2):
                                    kt = d0 + a
                                    last = (a == 2 * j2 + 1)
                                    sps = spool.tile([P, QCW], F32, tag="s")
                                    nc.tensor.matmul(
                                        sps[:, 0:256],
                                        kT[:, h, b, kt * P:(kt + 1) * P],
                                        qT[:, h, b, qs:qs + 256],
                                        start=True, stop=True,
                                    )
                                    et = epool.tile([P, 256], BF16, tag="e256")
                                    nc.scalar.activation(
                                        et[:], sps[:, 0:256], AF.Exp, scale=SCALE
                                    )
                                    aa = a - 2 * j2
                                    if aa >= 0:  # partially masked tile
                                        nc.vector.tensor_mul(
                                            et[:], et[:],
                                            masks_sb[:, aa, 0:256]
                                        )
                                    reg = slice(j2 * 256, j2 * 256 + 256)
                                    nc.tensor.matmul(
                                        ctxu_ps[:, reg],
                                        vN[:, b * KT_PER_B + kt,
                                           h * HD:(h + 1) * HD],
                                        et[:],
                                        start=(qc == 0 and a == 0), stop=last,
                                    )
                                    nc.tensor.matmul(
                                        sums_bc[:, reg], ones_sb[:], et[:],
                                        start=(qc == 0 and a == 0), stop=last,
                                    )
                            recip_bc = smpool.tile([P, QCW], F32, tag="recipbc")
                            nc.vector.reciprocal_approx_fast(recip_bc[:], sums_bc[:])
                            ctx = ctxpool.tile([P, QCW], BF16, tag="ctx")
                            nc.vector.tensor_mul(ctx[:], ctxu_ps[:], recip_bc[:])
                            ctx_tiles.append(ctx)

                        if prev is not None:
                            outproj(*prev, nts=(0, 1, 2, 3))
                        prev = (ctx_tiles, b, qc)
                outproj(*prev, nts=(0, 1, 2, 3))

    nc.compile()
    return nc


def _host_prep(x, wq, bq, wk, bk, wv, bv, wo):
    """Build the 8 per-core input maps."""
    x = np.asarray(x, dtype=np.float32)
    xT = np.ascontiguousarray(x.reshape(BN, C).T)  # [C, BN]
    x8T = xT.astype(NP_FP8)
    xbT = xT.astype(NP_BF16)

    m = np.zeros((4, P, QCW), dtype=np.float32)
    kl = np.arange(P)[:, None]
    ql = np.arange(QCW)[None, :]
    for a in range(4):
        m[a] = (ql >= (P * a + kl)).astype(np.float32)
    m = m.astype(NP_BF16)

    wq_f = np.asarray(wq, dtype=np.float32)
    wk_f = np.asarray(wk, dtype=np.float32)
    wv_f = np.asarray(wv, dtype=np.float32)
    wo_f = np.asarray(wo, dtype=np.float32)

    bq_f = np.asarray(bq, np.float32)
    bk_f = np.asarray(bk, np.float32)
    bv_f = np.asarray(bv, np.float32)

    in_maps = []
    for c in range(NCORES):
        e0 = c * E
        wqk = np.concatenate(
            [wq_f[e0:e0 + E, :].T, wk_f[e0:e0 + E, :].T], axis=1) * WSCALE
        bqkv = np.concatenate([
            bq_f[e0:e0 + E].reshape(HPC, P),
            bk_f[e0:e0 + E].reshape(HPC, P),
            bv_f[e0:e0 + E].reshape(HPC, P)], axis=0)
        in_maps.append({
            "x8T": x8T,
            "xbT": xbT,
            "wqk8": np.ascontiguousarray(wqk).astype(NP_FP8),
            "wvT": np.ascontiguousarray(wv_f[e0:e0 + E, :].T).astype(NP_BF16),
            "woT": np.ascontiguousarray(wo_f[:, e0:e0 + E].T).astype(NP_BF16),
            "bqkv": np.ascontiguousarray(bqkv),
            "masks": m,
            "ones_d": np.ones((P, P), dtype=NP_BF16),
            "ident_d": np.eye(P, dtype=np.float32),
        })
    return in_maps


def _ensure_ntff_hook_module():
    """run_bass_kernel_spmd(trace=True) imports antenv.axon_hooks; provide a
    stub (hook=None -> tracing skipped gracefully) if the module is absent."""
    try:
        import antenv.axon_hooks  # noqa: F401
    except ImportError:
        import sys
        import types
        try:
            import antenv
        except ImportError:
            return
        mod = types.ModuleType("antenv.axon_hooks")
        state = {"hook": None}
        mod.set_axon_ntff_profile_hook = lambda h: state.__setitem__("hook", h)
        mod.get_axon_ntff_profile_hook = lambda: state["hook"]
        sys.modules["antenv.axon_hooks"] = mod
        antenv.axon_hooks = mod


def kernel(**inputs):
    _ensure_ntff_hook_module()
    if "nc" not in _CACHE:
        _CACHE["nc"] = _build()
    nc = _CACHE["nc"]

    in_maps = _host_prep(
        inputs["x"], inputs["wq"], inputs["bq"], inputs["wk"], inputs["bk"],
        inputs["wv"], inputs["bv"], inputs["wo"],
    )

    res = bass_utils.run_bass_kernel_spmd(
        nc, in_maps, core_ids=list(range(NCORES)),
        trace=bool(os.environ.get("BASS_TRACE")),
    )
    _CACHE["last_result"] = res

    y = np.zeros((BN, C), dtype=np.float64)
    for c in range(NCORES):
        y += res.results[c]["yp"].astype(np.float64)
    y += np.asarray(inputs["bo"], dtype=np.float64)
    return y.astype(np.float32).reshape(B, N, C)
